# revision 21
# baseline (speedup 1.0000x reference)
"""Trainium2 Bass kernel for nn_MetricLoss (affine-invariant Riemannian metric loss).

Math: per row, S0 = sym2x2(prediction), M = sym2x2(actual) (SPD).
Row loss = ||log(M0^{-1/2} M M0^{-1/2})||_F^2 with M0 = exp(S0); it only
depends on the two generalized eigenvalues l1, l2 of det(M - l*M0) = 0:
    y = log(l1)^2 + log(l2)^2 = (L^2 + G^2)/2
    L = log(l1*l2) = log det M - tr(S0)
    G = log(l2/l1) = 2*arccosh(k),  k = s/(2 sqrt(p))
    s = tr(M0^{-1} M),  p = det M * exp(-tr S0)
with M0^{-1} = exp(-S0) evaluated in closed form for symmetric 2x2.
All elementwise: DVE + ACT + Pool, one activation table set (Exp/Ln/Square);
sqrt(x) = Exp(0.5*Ln(x)) so no table switches ever happen.

Sharding: pure data parallel, batch split across 8 cores; each core reduces
its shard to [128, 2*NT] partial sums (ACT accum_out); host sums and divides.
"""

import types

import numpy as np

import bass_rust
import concourse.mybir as mybir
from concourse import bacc, bass
from concourse.hw_specs import get_activation_tables
from concourse.tile import TileContext
from concourse.bass_utils import run_bass_kernel_spmd

N_CORES = 8
B_TOTAL = 1_048_576
P = 128
SHARD = B_TOTAL // N_CORES  # 131072 rows per core
CPT = SHARD // P            # 1024 rows per partition
NT = 2                      # chunks per core

F32 = mybir.dt.float32
OP = mybir.AluOpType
AF = mybir.ActivationFunctionType

INV_SQRT2 = float(1.0 / np.sqrt(2.0))
SQRT2 = float(np.sqrt(2.0))
GSQ_BIAS = float(-np.sqrt(2.0) * np.log(4.0))


def _patched_insert_act_table_loads(self):
    """Force the one table set that covers Exp+Ln+Square so the greedy
    per-instruction set selection can't thrash between per-func sets
    (each reload costs ~2.7us). Non-covering sets are passed with an empty
    func set, keeping list order so act_func_set_id indices stay valid."""
    has_activation = any(
        isinstance(i, mybir.InstActivation)
        for b in self.main_func.blocks
        for i in b.instructions
    )
    if not has_activation:
        return
    need = {AF.Exp, AF.Ln, AF.Square}
    tables = [
        (name, funcs if need <= funcs else set())
        for name, funcs in get_activation_tables(self.m.arch).items()
    ]
    bass_rust.insert_act_table_loads(self, tables)


def build(nt=NT):
    chunks = (CPT // nt,) * nt if isinstance(nt, int) else tuple(nt)
    assert sum(chunks) == CPT
    nt = len(chunks)
    nc = bacc.Bacc()
    nc.insert_act_table_loads = types.MethodType(_patched_insert_act_table_loads, nc)
    pred = nc.dram_tensor("pred", [P, CPT * 3], F32, kind="ExternalInput")
    act = nc.dram_tensor("act", [P, CPT * 3], F32, kind="ExternalInput")
    out = nc.dram_tensor("out", [P, 2 * nt], F32, kind="ExternalOutput")

    with TileContext(nc) as tc:
        wbufs = 1 if nt == 1 else 2
        with (
            tc.tile_pool(name="accp", bufs=1) as accp,
            tc.tile_pool(name="io", bufs=wbufs) as iop,
            tc.tile_pool(name="work", bufs=wbufs) as wp,
        ):
            acc = accp.tile([P, 2 * nt], F32)
            V, S, G = nc.vector, nc.scalar, nc.gpsimd

            coff = 0
            for t in range(nt):
                cc = chunks[t]
                pt = iop.tile([P, cc * 3], F32, tag="pt")
                at = iop.tile([P, cc * 3], F32, tag="at")
                nc.sync.dma_start(out=pt[:], in_=pred[:, coff * 3:(coff + cc) * 3])
                nc.sync.dma_start(out=at[:], in_=act[:, coff * 3:(coff + cc) * 3])
                coff += cc
                p3 = pt[:].rearrange("p (n c) -> p n c", c=3)
                a3 = at[:].rearrange("p (n c) -> p n c", c=3)
                a0, b0, c0 = p3[:, :, 0], p3[:, :, 1], p3[:, :, 2]
                A, Bc, C = a3[:, :, 0], a3[:, :, 1], a3[:, :, 2]

                def wt(name):
                    return wp.tile([P, cc], F32, tag=name, name=name)

                # prediction-only chain
                tau = wt("tau"); G.tensor_tensor(tau, a0, c0, OP.add)
                df = wt("df");   V.tensor_tensor(df, a0, c0, OP.subtract)
                t4 = wt("t4");   S.activation(t4, df, AF.Square)
                t3 = wt("t3");   S.activation(t3, b0, AF.Square, scale=2.0)
                r2 = wt("r2");   V.scalar_tensor_tensor(r2, t4, 1e-30, t3, OP.max, OP.add)
                lr = wt("lr");   S.activation(lr, r2, AF.Ln)
                R = wt("R");     S.activation(R, lr, AF.Exp, scale=0.5)
                rinv = wt("rinv"); S.activation(rinv, lr, AF.Exp, scale=-0.5)
                d1 = wt("d1");   V.tensor_tensor(d1, R, tau, OP.subtract)
                d2 = wt("d2");   G.tensor_tensor(d2, R, tau, OP.add)
                E1 = wt("E1");   S.activation(E1, d1, AF.Exp, scale=0.5)
                E2 = wt("E2");   S.activation(E2, d2, AF.Exp, scale=-0.5)
                E12 = wt("E12"); S.activation(E12, tau, AF.Exp, scale=-1.0)
                # actual-only chain
                SA = wt("SA");   G.tensor_tensor(SA, A, C, OP.add)
                DA = wt("DA");   V.tensor_tensor(DA, A, C, OP.subtract)
                z1 = wt("z1");   G.tensor_tensor(z1, A, C, OP.mult)
                z2 = wt("z2");   S.activation(z2, Bc, AF.Square)
                d4 = wt("d4");   V.tensor_tensor(d4, z1, z2, OP.subtract)
                # cross terms
                x1 = wt("x1");   V.tensor_tensor(x1, df, DA, OP.mult)
                x2 = wt("x2");   V.scalar_tensor_tensor(x2, Bc, 4.0, b0, OP.mult, OP.mult)
                N2 = wt("N2");   V.tensor_tensor(N2, x1, x2, OP.add)
                w = wt("w");     V.tensor_tensor(w, N2, rinv, OP.mult)
                u1 = wt("u1");   V.tensor_tensor(u1, SA, w, OP.subtract)
                u2 = wt("u2");   G.tensor_tensor(u2, SA, w, OP.add)
                P1 = wt("P1");   V.tensor_tensor(P1, u1, E1, OP.mult)
                P2 = wt("P2");   V.tensor_tensor(P2, u2, E2, OP.mult)
                s2x = wt("s2x"); V.tensor_tensor(s2x, P1, P2, OP.add)
                m2 = wt("m2");   V.tensor_tensor(m2, d4, E12, OP.mult)
                Lp = wt("Lp");   S.activation(Lp, m2, AF.Ln)
                pinvh = wt("pinvh"); S.activation(pinvh, Lp, AF.Exp, scale=-0.5)
                khat = wt("khat"); V.tensor_tensor(khat, s2x, pinvh, OP.mult)
                kkB = wt("kkB"); S.activation(kkB, khat, AF.Square)
                kkm = wt("kkm"); V.tensor_scalar(kkm, kkB, 16.0, 0.0, OP.subtract, OP.max)
                lnB = wt("lnB"); S.activation(lnB, kkm, AF.Ln)
                sqB = wt("sqB"); S.activation(sqB, lnB, AF.Exp, scale=0.5)
                argB = wt("argB"); V.tensor_tensor(argB, khat, sqB, OP.add)
                G2B = wt("G2B"); S.activation(G2B, argB, AF.Ln, scale=0.25)
                Lsq = wt("Lsq")
                S.activation(Lsq, Lp, AF.Square, scale=INV_SQRT2,
                             accum_out=acc[:, 2 * t:2 * t + 1])
                Gsq = wt("Gsq")
                S.activation(Gsq, G2B, AF.Square, scale=SQRT2,
                             accum_out=acc[:, 2 * t + 1:2 * t + 2])

            nc.sync.dma_start(out=out[:], in_=acc[:])

    nc.finalize()
    return nc


_CACHED = {}


def _get_nc(nt=NT):
    key = nt if isinstance(nt, int) else tuple(nt)
    if key not in _CACHED:
        _CACHED[key] = build(nt)
    return _CACHED[key]


class _Runner:
    """Cached-jit SPMD runner mirroring bass2jax.run_bass_via_pjrt's
    multi-core path, so repeated executions reuse the compiled program."""

    def __init__(self, nt=NT):
        import jax
        from jax.sharding import Mesh, PartitionSpec
        from jax.experimental.shard_map import shard_map
        from concourse import bass2jax

        self.jax = jax
        nc = _get_nc(nt)
        self.nc = nc
        self.nt = nt if isinstance(nt, int) else len(nt)
        bass2jax.install_neuronx_cc_hook()

        partition_name = (nc.partition_id_tensor.name
                          if nc.partition_id_tensor else None)
        in_names, out_names, out_avals, zero_outs = [], [], [], []
        for alloc in nc.m.functions[0].allocations:
            if not isinstance(alloc, mybir.MemoryLocationSet):
                continue
            name = alloc.memorylocations[0].name
            if alloc.kind == "ExternalInput":
                if name != partition_name:
                    in_names.append(name)
            elif alloc.kind == "ExternalOutput":
                shape = tuple(alloc.tensor_shape)
                dtype = mybir.dt.np(alloc.dtype)
                out_names.append(name)
                out_avals.append(jax.core.ShapedArray(shape, dtype))
                zero_outs.append(np.zeros(shape, dtype))
        self.in_names = list(in_names)
        self.out_names = out_names
        self.zero_outs = zero_outs
        n_params = len(in_names)
        n_outs = len(out_avals)
        all_names = in_names + out_names
        if partition_name is not None:
            all_names.append(partition_name)

        def _body(*args):
            operands = list(args)
            if partition_name is not None:
                operands.append(bass2jax.partition_id_tensor())
            outs = bass2jax._bass_exec_p.bind(
                *operands,
                out_avals=tuple(out_avals),
                in_names=tuple(all_names),
                out_names=tuple(out_names),
                lowering_input_output_aliases=(),
                sim_require_finite=True,
                sim_require_nnan=True,
                nc=nc,
            )
            return tuple(outs)

        devices = jax.devices()[:N_CORES]
        mesh = Mesh(np.asarray(devices), ("core",))
        in_specs = (PartitionSpec("core"),) * (n_params + n_outs)
        out_specs = (PartitionSpec("core"),) * n_outs
        donate = tuple(range(n_params, n_params + n_outs))
        self.sharded = jax.jit(
            shard_map(_body, mesh=mesh, in_specs=in_specs,
                      out_specs=out_specs, check_rep=False),
            donate_argnums=donate, keep_unused=True,
        )
        self.n_params = n_params
        self.n_outs = n_outs

    def concat_inputs(self, in_maps):
        return [
            np.concatenate([np.asarray(m[name]) for m in in_maps], axis=0)
            for name in self.in_names
        ]

    def call_raw(self, concat_in):
        zeros = [np.zeros((N_CORES * z.shape[0], *z.shape[1:]), z.dtype)
                 for z in self.zero_outs]
        return self.sharded(*concat_in, *zeros)

    def __call__(self, in_maps):
        out_arrs = self.call_raw(self.concat_inputs(in_maps))
        per_core_rows = self.zero_outs[0].shape[0]
        out0 = np.asarray(out_arrs[0]).reshape(N_CORES, per_core_rows, -1)
        return [{self.out_names[0]: out0[c]} for c in range(N_CORES)]


_RUNNERS = {}


def _get_runner(nt=NT):
    key = nt if isinstance(nt, int) else tuple(nt)
    if key not in _RUNNERS:
        _RUNNERS[key] = _Runner(nt)
    return _RUNNERS[key]


def _in_maps(prediction, actual):
    pred = np.ascontiguousarray(prediction, dtype=np.float32)
    act = np.ascontiguousarray(actual, dtype=np.float32)
    maps = []
    for c in range(N_CORES):
        ps = pred[c * SHARD:(c + 1) * SHARD].reshape(P, CPT * 3)
        as_ = act[c * SHARD:(c + 1) * SHARD].reshape(P, CPT * 3)
        maps.append({"pred": ps, "act": as_})
    return maps


def run(prediction, actual, nt=NT):
    runner = _get_runner(nt)
    results = runner(_in_maps(prediction, actual))
    total = 0.0
    for om in results:
        total += om["out"].sum(dtype=np.float64)
    return np.float32(total / B_TOTAL), results


def kernel(prediction, actual):
    value, _ = run(prediction, actual)
    return value
